# revision 25
# baseline (speedup 1.0000x reference)
"""DLRM (nn_DLRM_RPC) Trainium2 Bass kernel — v3.

Data-parallel over batch across 8 NeuronCores; embedding tables replicated
per core (bf16), no collectives.

Per core (2048 samples, 4 tiles of 512). Within a tile, samples are indexed
n = 128*s + g (g = Gram group of 4, s = slot within group, g = 4*sg + g2);
the host permutes the index rows so gather slot p = 16*G'l + 4*s + g2 of
chunk c holds sample n = 128*s + 32*c + 4*G'l + g2, which makes every
on-chip copy a legal 3-free-dim AP, burst-contiguous on both sides:

  - Eall[d, 512*sg + 128*s + 4*t + g2]  (t = slot: 0 = bottom-MLP x,
    1..26 = tables, 27..31 junk).  Phase-A PSUM drains are single 4-free-dim
    copies with 4*ntb-element contiguous bursts, split Vector/Scalar.
  - The Gram lhsT/rhs for group g is then the single-free-dim stride-4 AP
    [512*sg + g2 + 4*q], whose position q = 32*s + t gives output rows at
    32*s + t — 32-aligned as both matmul-AP and partition-base rules
    require.  The scramble is plain 32-aligned partition-shifted copies
    with contiguous 256B destination runs (dst cols = 128*s + g), split
    across Vector and GpSimd.
  - Emission is software-pipelined (AB(n+1) before E(n)) because engine
    queues are FIFO: PE transposes of the next tile run while this tile's
    scramble finishes, so the PE never head-of-line blocks.

All matmuls bf16 with fp32 PSUM accumulation.
"""

import os
import sys

import numpy as np

for _p in ("/opt/trn_rl_repo",):
    if _p not in sys.path and os.path.isdir(_p):
        sys.path.insert(0, _p)

import ml_dtypes

import concourse.bass as bass
import concourse.bacc as bacc
import concourse.mybir as mybir
import concourse.tile as tile
from concourse import bass_utils
from concourse.bass_interp import get_hw_module
from concourse.masks import make_identity

BF16 = ml_dtypes.bfloat16
F32 = np.float32

N_CORES = 8
B = 16384
SPC = B // N_CORES        # samples per core: 2048
NT = 27                   # slots: x + 26 tables
NE = 26
VOCAB = 50000
D = 128
SW = 512                  # super-group block width: 4 groups interleaved
TS = 512                  # samples per tile
NTILES = SPC // TS        # 4
G = TS // 4               # groups per tile: 128
NSG = G // 4              # super-groups per tile: 32
CH = TS // 128            # 128-sample chunks per tile: 4
TGRP = [(0, 7), (7, 7), (14, 6), (20, 6)]   # (first table, count) per PSUM bank

LI, LJ = np.tril_indices(NT, -1)

_dt_bf16 = mybir.dt.bfloat16
_dt_f32 = mybir.dt.float32
_dt_i32 = mybir.dt.int32

_CACHE = {}


def _emit(tc, t):
    from contextlib import ExitStack

    nc = tc.nc
    Relu = mybir.ActivationFunctionType.Relu
    Sigmoid = mybir.ActivationFunctionType.Sigmoid
    Copy = mybir.ActivationFunctionType.Copy

    with ExitStack() as ctx:
        sb = ctx.enter_context(tc.tile_pool(name="sb", bufs=1))
        db = ctx.enter_context(tc.tile_pool(name="db", bufs=2))
        hb = ctx.enter_context(tc.tile_pool(name="hb", bufs=1))
        mmps = ctx.enter_context(tc.tile_pool(name="mmps", bufs=3, space="PSUM"))
        grps = ctx.enter_context(tc.tile_pool(name="grps", bufs=2, space="PSUM"))
        w3ps = ctx.enter_context(tc.tile_pool(name="w3ps", bufs=1, space="PSUM"))

        def load(name, shape, dtype=_dt_bf16):
            tl = sb.tile(shape, dtype, name=name)
            nc.sync.dma_start(tl[:], t[name][:])
            return tl

        dxt = load("dxt", [16, SPC])
        bw0 = load("bw0", [16, 512])
        bb0 = load("bb0", [128, 4], _dt_f32)
        bw1 = load("bw1", [128, 4 * 256])
        bb1 = load("bb1", [128, 2], _dt_f32)
        bw2 = load("bw2", [128, 2 * 128])
        bb2 = load("bb2", [128, 1], _dt_f32)
        w0x = load("w0x", [128, 1024])
        wz = load("wz", [128, 7 * 1024])
        tb0 = load("tb0", [128, 8], _dt_f32)
        w1 = load("w1", [128, 8 * 1024])
        tb1 = load("tb1", [128, 8], _dt_f32)
        w2 = load("w2", [128, 8 * 512])
        tb2 = load("tb2", [128, 4], _dt_f32)
        w3 = load("w3", [128, 4])
        tb3 = load("tb3", [1, 1], _dt_f32)

        zsb = sb.tile([128, SW * NSG], _dt_bf16)
        zsbv = zsb[:].rearrange("p (sg cc g2) -> p sg cc g2", cc=128, g2=4)
        zstk = [sb.tile([128, TS], _dt_bf16, name=f"zstk{q}") for q in range(7)]
        for q in range(7):
            nc.vector.memset(zstk[q][:], 0.0)

        def phase_ab(n):
            """Gather + transpose into a fresh Eall; bottom MLP -> x."""
            eall = db.tile([128, SW * NSG], _dt_bf16, name="eall")
            eb = eall[:]
            pstep = eb.ap[0]
            for c in range(CH):
                C = CH * n + c
                idxt = db.tile([128, NE], _dt_i32, name="idxt")
                nc.sync.dma_start(idxt[:], t["idx"][128 * C:128 * (C + 1), :])
                esm = db.tile([128, NE * D], _dt_bf16, name="esm")
                nc.gpsimd.indirect_dma_start(
                    out=esm[:], out_offset=None,
                    in_=t["tbl"][:],
                    in_offset=bass.IndirectOffsetOnAxis(ap=idxt[:], axis=0),
                )
                esmT = hb.tile([128, NE * D], _dt_bf16, name="esmT")
                for ti in range(NE):
                    nc.sync.dma_start_transpose(
                        esmT[:, 128 * ti:128 * (ti + 1)],
                        esm[:, 128 * ti:128 * (ti + 1)])
                et = esmT[:]
                for bi, (tb_, ntb) in enumerate(TGRP):
                    src = bass.AP(et.tensor, et.offset + 128 * tb_,
                                  [et.ap[0], [4, 32], [128, ntb], [1, 4]])
                    dst = bass.AP(eb.tensor,
                                  eb.offset + SW * 8 * c + 4 * (tb_ + 1),
                                  [pstep, [128, 32], [4, ntb], [1, 4]])
                    if bi < 2:
                        nc.vector.tensor_copy(dst, src)
                    else:
                        nc.scalar.activation(dst, src, Copy)

            # bottom MLP -> xbuf (n-order) + Eall slot 0
            h0 = hb.tile([128, 4 * 512], _dt_bf16, name="h0")
            for m in range(4):
                ps = mmps.tile([128, 512], _dt_f32, name="mm", tag="mm")
                nc.tensor.matmul(ps[:], bw0[:, 128 * m:128 * (m + 1)],
                                 dxt[:, TS * n:TS * (n + 1)],
                                 start=True, stop=True)
                nc.scalar.activation(h0[:, 512 * m:512 * (m + 1)], ps[:],
                                     Relu, bias=bb0[:, m:m + 1])
            h1b = hb.tile([128, 2 * 512], _dt_bf16, name="h1b")
            for m in range(2):
                ps = mmps.tile([128, 512], _dt_f32, name="mm", tag="mm")
                for k in range(4):
                    nc.tensor.matmul(
                        ps[:], bw1[:, 256 * k + 128 * m:256 * k + 128 * (m + 1)],
                        h0[:, 512 * k:512 * (k + 1)],
                        start=(k == 0), stop=(k == 3))
                nc.scalar.activation(h1b[:, 512 * m:512 * (m + 1)], ps[:],
                                     Relu, bias=bb1[:, m:m + 1])
            ps = mmps.tile([128, 512], _dt_f32, name="mm", tag="mm")
            for k in range(2):
                nc.tensor.matmul(ps[:], bw2[:, 128 * k:128 * (k + 1)],
                                 h1b[:, 512 * k:512 * (k + 1)],
                                 start=(k == 0), stop=(k == 1))
            xbuf = db.tile([128, 512], _dt_bf16, name="xbuf")
            nc.scalar.activation(xbuf[:], ps[:], Relu, bias=bb2[:, 0:1])
            xa = xbuf[:]
            nc.gpsimd.tensor_copy(
                bass.AP(eb.tensor, eb.offset,
                        [pstep, [128, 4], [512, NSG], [1, 4]]),
                bass.AP(xa.tensor, xa.offset,
                        [xa.ap[0], [128, 4], [4, NSG], [1, 4]]))
            return eall, xbuf

        def phase_c(n, eall):
            """Gram matmuls -> CAST to zsb."""
            eb = eall[:]
            pstep = eb.ap[0]
            for sg in range(NSG):
                bank = grps.tile([128, 512], _dt_f32, name="grb", tag="gr")
                ba = bank[:]
                for g2 in range(4):
                    gap = bass.AP(eb.tensor, eb.offset + SW * sg + g2,
                                  [pstep, [4, 128]])
                    out = bass.AP(ba.tensor, ba.offset + g2,
                                  [ba.ap[0], [4, 128]])
                    nc.tensor.matmul(out, gap, gap, start=True, stop=True)
                eng = nc.vector if (sg % 2 == 0) else nc.scalar
                if sg % 2 == 0:
                    eng.tensor_copy(zsb[:, SW * sg:SW * (sg + 1)], bank[:, :])
                else:
                    eng.activation(zsb[:, SW * sg:SW * (sg + 1)], bank[:, :],
                                   Copy)

        def phase_d(n):
            """Scramble zsb into the 7 K-stacked zstk tiles."""
            ci = 0
            for s in range(4):
                for i in range(1, NT):
                    q, u = i // 4, i % 4
                    eng = nc.vector if (ci % 2 == 0) else nc.gpsimd
                    ci += 1
                    eng.tensor_copy(
                        zstk[q][32 * u:32 * u + NT, 128 * s:128 * (s + 1)],
                        zsbv[32 * s:32 * s + NT, :, 32 * s + i, :])

        def phase_e(n, xbuf):
            """Top MLP + output DMA."""
            h1t = hb.tile([128, 8 * 512], _dt_bf16, name="h1t")
            for m in range(8):
                ps = mmps.tile([128, 512], _dt_f32, name="mm", tag="mm")
                nc.tensor.matmul(ps[:], w0x[:, 128 * m:128 * (m + 1)],
                                 xbuf[:], start=True, stop=False)
                for q in range(7):
                    nc.tensor.matmul(
                        ps[:], wz[:, 1024 * q + 128 * m:1024 * q + 128 * (m + 1)],
                        zstk[q][:], start=False, stop=(q == 6))
                nc.scalar.activation(h1t[:, 512 * m:512 * (m + 1)], ps[:],
                                     Relu, bias=tb0[:, m:m + 1])
            h2t = hb.tile([128, 8 * 512], _dt_bf16, name="h2t")
            for m in range(8):
                ps = mmps.tile([128, 512], _dt_f32, name="mm", tag="mm")
                for k in range(8):
                    nc.tensor.matmul(
                        ps[:], w1[:, 1024 * k + 128 * m:1024 * k + 128 * (m + 1)],
                        h1t[:, 512 * k:512 * (k + 1)],
                        start=(k == 0), stop=(k == 7))
                nc.scalar.activation(h2t[:, 512 * m:512 * (m + 1)], ps[:],
                                     Relu, bias=tb1[:, m:m + 1])
            h3t = hb.tile([128, 4 * 512], _dt_bf16, name="h3t")
            for m in range(4):
                ps = mmps.tile([128, 512], _dt_f32, name="mm", tag="mm")
                for k in range(8):
                    nc.tensor.matmul(
                        ps[:], w2[:, 512 * k + 128 * m:512 * k + 128 * (m + 1)],
                        h2t[:, 512 * k:512 * (k + 1)],
                        start=(k == 0), stop=(k == 7))
                nc.scalar.activation(h3t[:, 512 * m:512 * (m + 1)], ps[:],
                                     Relu, bias=tb2[:, m:m + 1])
            ps3 = w3ps.tile([1, 512], _dt_f32, name="w3p", tag="w3")
            for k in range(4):
                nc.tensor.matmul(ps3[:], w3[:, k:k + 1],
                                 h3t[:, 512 * k:512 * (k + 1)],
                                 start=(k == 0), stop=(k == 3))
            outsb = db.tile([1, 512], _dt_f32, name="outsb")
            nc.scalar.activation(outsb[:], ps3[:], Sigmoid, bias=tb3[0:1, 0:1])
            nc.sync.dma_start(t["out"][n:n + 1, :], outsb[:])

        # software-pipelined emission: AB/C(n+1) before E(n), D(n+1) after —
        # so scramble(n+1) on V/G overlaps L1-3(n) + AB/C(n+2) on PE.
        eall, xbuf = phase_ab(0)
        phase_c(0, eall)
        phase_d(0)
        prev_x = xbuf
        for n in range(1, NTILES):
            eall, xbuf = phase_ab(n)
            phase_c(n, eall)
            phase_e(n - 1, prev_x)
            phase_d(n)
            prev_x = xbuf
        phase_e(NTILES - 1, prev_x)


def _build():
    if "nc" in _CACHE:
        return _CACHE["nc"]
    nc = bacc.Bacc("TRN2", target_bir_lowering=False, debug=False,
                   num_devices=N_CORES)
    t = {}

    def dram(name, shape, dt, kind="ExternalInput"):
        t[name] = nc.dram_tensor(name, shape, dt, kind=kind).ap()

    dram("tbl", [NE * VOCAB, D], _dt_bf16)
    dram("idx", [SPC, NE], _dt_i32)
    dram("dxt", [16, SPC], _dt_bf16)
    dram("bw0", [16, 512], _dt_bf16)
    dram("bb0", [128, 4], _dt_f32)
    dram("bw1", [128, 4 * 256], _dt_bf16)
    dram("bb1", [128, 2], _dt_f32)
    dram("bw2", [128, 2 * 128], _dt_bf16)
    dram("bb2", [128, 1], _dt_f32)
    dram("w0x", [128, 1024], _dt_bf16)
    dram("wz", [128, 7 * 1024], _dt_bf16)
    dram("tb0", [128, 8], _dt_f32)
    dram("w1", [128, 8 * 1024], _dt_bf16)
    dram("tb1", [128, 8], _dt_f32)
    dram("w2", [128, 8 * 512], _dt_bf16)
    dram("tb2", [128, 4], _dt_f32)
    dram("w3", [128, 4], _dt_bf16)
    dram("tb3", [1, 1], _dt_f32)
    dram("out", [NTILES, TS], _dt_f32, kind="ExternalOutput")

    with tile.TileContext(nc) as tc:
        _emit(tc, t)
    nc.compile()

    _CACHE["nc"] = nc
    return nc


def _ktile(w, kt, m):
    """[K, M] -> [128, (K//128) * M] with column kt*M + mm = w[128*kt + p, mm]."""
    K, Mo = w.shape
    return np.ascontiguousarray(
        w.reshape(K // 128, 128, Mo).transpose(1, 0, 2).reshape(128, -1))


def _shared_inputs(inputs):
    emb = np.asarray(inputs["emb_tables"])
    tbl = np.ascontiguousarray(
        emb.astype(BF16).reshape(NE * VOCAB, D))

    sh = {"tbl": tbl}
    sh["bw0"] = np.zeros((16, 512), BF16)
    sh["bw0"][:13] = np.asarray(inputs["bot_W0"]).astype(BF16)
    sh["bb0"] = np.asarray(inputs["bot_b0"]).astype(F32).reshape(4, 128).T.copy()
    sh["bw1"] = _ktile(np.asarray(inputs["bot_W1"]).astype(BF16), 4, 256)
    sh["bb1"] = np.asarray(inputs["bot_b1"]).astype(F32).reshape(2, 128).T.copy()
    sh["bw2"] = _ktile(np.asarray(inputs["bot_W2"]).astype(BF16), 2, 128)
    sh["bb2"] = np.asarray(inputs["bot_b2"]).astype(F32).reshape(1, 128).T.copy()

    w0 = np.asarray(inputs["top_W0"]).astype(F32)
    sh["w0x"] = w0[:128].astype(BF16)
    wgrid = np.zeros((NT, NT, 1024), F32)
    wgrid[LI, LJ] = w0[128:479]
    wz4 = np.zeros((7, 128, 1024), F32)
    for i in range(NT):
        q, u = i // 4, i % 4
        wz4[q, 32 * u:32 * u + NT] = wgrid[i]
    sh["wz"] = np.ascontiguousarray(
        wz4.transpose(1, 0, 2).reshape(128, 7 * 1024)).astype(BF16)
    sh["tb0"] = np.asarray(inputs["top_b0"]).astype(F32).reshape(8, 128).T.copy()
    sh["w1"] = _ktile(np.asarray(inputs["top_W1"]).astype(BF16), 8, 1024)
    sh["tb1"] = np.asarray(inputs["top_b1"]).astype(F32).reshape(8, 128).T.copy()
    sh["w2"] = _ktile(np.asarray(inputs["top_W2"]).astype(BF16), 8, 512)
    sh["tb2"] = np.asarray(inputs["top_b2"]).astype(F32).reshape(4, 128).T.copy()
    sh["w3"] = _ktile(np.asarray(inputs["top_W3"]).astype(BF16), 4, 1)
    sh["tb3"] = np.asarray(inputs["top_b3"]).astype(F32).reshape(1, 1)
    return sh


def _gather_perm():
    """perm[r] = within-core sample index held by gather slot r."""
    r = np.arange(SPC)
    T, rr = r // TS, r % TS
    c, p = rr // 128, rr % 128
    n = 128 * ((p // 4) % 4) + 32 * c + 4 * (p // 16) + (p % 4)
    return TS * T + n


def _in_maps(inputs):
    sh = _shared_inputs(inputs)
    idx = np.asarray(inputs["indices"]).astype(np.int64)      # [26, B]
    gidx = (idx + (np.arange(NE) * VOCAB)[:, None]).astype(np.int32)
    dx = np.asarray(inputs["dense_x"]).astype(F32)            # [B, 13]
    perm = _gather_perm()
    maps = []
    for core in range(N_CORES):
        sl = slice(SPC * core, SPC * (core + 1))
        m = dict(sh)
        m["idx"] = np.ascontiguousarray(gidx[:, sl][:, perm].T)   # [2048, 26]
        dxt = np.zeros((16, SPC), BF16)
        dxt[:13] = dx[sl].T.astype(BF16)
        m["dxt"] = dxt
        maps.append(m)
    return maps


def _run(inputs, trace=False):
    nc = _build()
    maps = _in_maps(inputs)
    old_m = nc.m
    nc.m = _CACHE.setdefault("hwm", get_hw_module(nc.m))
    try:
        res = bass_utils.run_bass_kernel_spmd(
            nc, maps, core_ids=list(range(N_CORES)), trace=trace)
    finally:
        nc.m = old_m
    out = np.concatenate([r["out"].reshape(-1) for r in res.results])
    return out.astype(F32).reshape(B, 1), res


def kernel(**inputs):
    out, _ = _run(inputs, trace=False)
    return out
